# revision 61
# baseline (speedup 1.0000x reference)
"""GQA attention (RoPE + ALiBi + causal) on 8 trn2 NeuronCores.

Sharding: core c -> batch b = c//4, kv-group g = c%4 (4 q-heads + 1 kv-head
per core, column-sharded Wq/Wk/Wv, row-sharded Wo; host sums the 4 partial
Wo outputs per batch).

Optimizations vs the original fp32r version (638us -> ~316us):
- all matmul operands bf16 (host-cast; PSUM accumulate stays f32), bf16 out
  (norm rel err ~5e-3, well under the 2e-2 gate)
- host pre-packs every dram tensor partition-major so DMA packets are >=1KB
  contiguous per partition (256B-chunk weight loads were the startup gate)
- x loaded once per block-pair, resident in SBUF for all 6 projection matmuls
  per c-tile; first c-tiles split per-block so the first matmuls start sooner
- cross-block software pipeline: attention(bk), whose inner loop is exp-gated
  on the scalar engine, absorbs the previous block's Wo tiles and the next
  block's k projection as drip-fed tensor work (front-loaded so the softmax
  oh-chain isn't queued behind trailing copies); v/q0/q1-3 passes + ropes run
  between attentions, keeping PSUM <= 8 banks throughout
- attention inner loop software-pipelined (s[j+1..2] issued before cs/ot[j])
- causal diagonal tiles shortened: the dead below-block region is never
  computed (matmul/exp run on [:, delta*128:] only), and the causal mask add
  is a single [128,128] triangle block on the diagonal sub-tile
- softmax denominator via a ones-vector matmul accumulated in PSUM; 1/cs via
  reciprocal_approx_fast (plain vector reciprocal on a [1,512] AP is 3.3us)
- per-query ALiBi term dropped (softmax-invariant); per-key term + block
  offset ride the exp activation's per-partition bias, so no row-max pass
- RoPE rotate-half done as a 213ns f32r PE permutation matmul (exact 0/1
  matrix) instead of a 128KB SBUF->SBUF DMA whose ~10us transfer latency
  was exposed at block seams
- all DMA issued from the sync queue (gpsimd-issued DMA measured slower);
  final-block output writes split small so the tail drain is short
"""
import sys

if '/opt/trn_rl_repo' not in sys.path:
    sys.path.insert(0, '/opt/trn_rl_repo')

import numpy as np
import ml_dtypes

BF = ml_dtypes.bfloat16

B, T, D = 2, 2048, 2048
H, KV = 16, 4
HD = D // H          # 128
NREP = H // KV       # 4
KVD = 512            # per-core q width (4 heads x 128)
P = 128
TB = 512             # t-block
NBLK = T // TB       # 4
NC = D // P          # 16 contraction tiles
NJ = T // P          # 16 key tiles
ALIBI_W = 0.1
SCALE = (1.0 - ALIBI_W) / np.sqrt(np.float32(HD))

_cache = {}


def _build():
    from concourse import bacc, mybir
    from concourse.tile import TileContext

    F32 = mybir.dt.float32
    BF16 = mybir.dt.bfloat16
    FR = mybir.dt.float32r
    EXP = mybir.ActivationFunctionType.Exp

    nc = bacc.Bacc()
    xp = nc.declare_dram_parameter("xp", [P, NC * T], BF16, isOutput=False)
    wqp = nc.declare_dram_parameter("wqp", [P, NC * KVD], BF16, isOutput=False)
    wkp = nc.declare_dram_parameter("wkp", [P, NC * P], BF16, isOutput=False)
    wvp = nc.declare_dram_parameter("wvp", [P, NC * P], BF16, isOutput=False)
    wop = nc.declare_dram_parameter("wop", [P, NREP * D], BF16, isOutput=False)
    cosT = nc.declare_dram_parameter("cosT", [P, T], F32, isOutput=False)
    sinT = nc.declare_dram_parameter("sinT", [P, T], F32, isOutput=False)
    cb = nc.declare_dram_parameter("cb", [P, NREP * NBLK * NJ], F32, isOutput=False)
    mtri = nc.declare_dram_parameter("mtri", [P, P], F32, isOutput=False)
    onesc = nc.declare_dram_parameter("onesc", [P, 1], BF16, isOutput=False)
    idin = nc.declare_dram_parameter("idin", [P, P], F32, isOutput=False)
    swm = nc.declare_dram_parameter("swm", [P, P], F32, isOutput=False)
    out = nc.declare_dram_parameter("out", [T, D], BF16, isOutput=True)

    wq_r = wqp.rearrange("p (c n) -> p c n", n=KVD)
    wk_r = wkp.rearrange("p (c n) -> p c n", n=P)
    wv_r = wvp.rearrange("p (c n) -> p c n", n=P)
    wo_r = wop.rearrange("p (h e) -> p h e", e=D)
    x_r = xp.rearrange("p (c t) -> p c t", t=T)

    with TileContext(nc) as tc:
        with (
            tc.tile_pool(name="const", bufs=1) as cpool,
            tc.tile_pool(name="kv", bufs=1) as kvpool,
            tc.tile_pool(name="xin", bufs=2) as xpool,
            tc.tile_pool(name="work", bufs=2) as wpool,
            tc.tile_pool(name="qt", bufs=8) as qpool,
            tc.tile_pool(name="pt", bufs=4) as ptpool,
            tc.tile_pool(name="oh", bufs=8) as opool,
            tc.tile_pool(name="ysb", bufs=4) as ypool,
            tc.tile_pool(name="small", bufs=2) as spool,
            tc.tile_pool(name="ps", bufs=1, space="PSUM") as pss,
        ):
            # ---- resident constants (DMA order = need order) ----
            wk_sb = cpool.tile([P, NC, P], BF16)
            wv_sb = cpool.tile([P, NC, P], BF16)
            wq_sb = cpool.tile([P, NC, KVD], BF16)
            wo_sb = cpool.tile([P, NREP, D], BF16)
            cos_sb = cpool.tile([P, T], F32)
            sin_sb = cpool.tile([P, T], F32)
            cb_sb = cpool.tile([P, NREP * NBLK * NJ], F32)
            mtri_sb = cpool.tile([P, P], F32)
            ones_sb = cpool.tile([P, 1], BF16)
            id_sb = cpool.tile([P, P], F32)
            swm_sb = cpool.tile([P, P], FR)   # rotate-half permutation (lhsT)
            kT_sb = kvpool.tile([P, T], BF16)        # roped K, [d, s]
            v_sb = kvpool.tile([P, NJ, P], BF16)     # V tiles, [s, j, d']

            x_tiles = {}

            def load_x_cols(g, c_lo, c_hi):
                # full-width c-tiles keep 2KB DMA packets; issuing only half
                # the c-range at a time leaves DMA engines free for the
                # latency-critical rope-swap transfers
                if g not in x_tiles:
                    x_tiles[g] = xpool.tile([P, NC, 2 * TB], BF16, tag="x",
                                            name=f"x{g}")
                for c in range(c_lo, c_hi):
                    nc.sync.dma_start(out=x_tiles[g][:, c],
                                      in_=x_r[:, c, g * 2 * TB:(g + 1) * 2 * TB])

            # startup DMA order = need order: x c-tiles interleaved with the
            # weight chunks the first projection pass reads alongside them
            x0 = xpool.tile([P, NC, 2 * TB], BF16, tag="x", name="x0")
            x_tiles[0] = x0
            for c4 in range(4):
                for c in range(c4 * 4, c4 * 4 + 4):
                    if c4 == 0:
                        # first 4 c-tiles: block-0 slice only, split across
                        # two DMA engines by partition halves so the very
                        # first matmuls start ~4us earlier
                        nc.sync.dma_start(out=x0[0:64, c, 0:TB],
                                          in_=x_r[0:64, c, 0:TB])
                        nc.sync.dma_start(out=x0[64:P, c, 0:TB],
                                          in_=x_r[64:P, c, 0:TB])
                    else:
                        nc.sync.dma_start(out=x0[:, c], in_=x_r[:, c, 0:2 * TB])
                if c4 == 0:
                    nc.sync.dma_start(out=wk_sb[0:64, 0:4], in_=wk_r[0:64, 0:4])
                    nc.sync.dma_start(out=wk_sb[64:P, 0:4], in_=wk_r[64:P, 0:4])
                else:
                    nc.sync.dma_start(out=wk_sb[:, c4 * 4:(c4 + 1) * 4],
                                      in_=wk_r[:, c4 * 4:(c4 + 1) * 4])
                nc.sync.dma_start(out=wv_sb[:, c4 * 4:(c4 + 1) * 4],
                                  in_=wv_r[:, c4 * 4:(c4 + 1) * 4])
                nc.sync.dma_start(out=wq_sb[:, c4 * 4:(c4 + 1) * 4],
                                  in_=wq_r[:, c4 * 4:(c4 + 1) * 4])
            nc.sync.dma_start(out=cos_sb[:, 0:TB], in_=cosT[:, 0:TB])
            nc.sync.dma_start(out=sin_sb[:, 0:TB], in_=sinT[:, 0:TB])
            nc.sync.dma_start(out=cb_sb, in_=cb[:, :])
            nc.sync.dma_start(out=mtri_sb, in_=mtri[:, :])
            nc.sync.dma_start(out=ones_sb, in_=onesc[:, :])
            nc.sync.dma_start(out=id_sb, in_=idin[:, :])
            nc.sync.dma_start(out=swm_sb, in_=swm[:, :].bitcast(FR))
            for s4 in range(1, 4):
                nc.sync.dma_start(out=cos_sb[:, s4 * TB:(s4 + 1) * TB],
                                  in_=cosT[:, s4 * TB:(s4 + 1) * TB])
                nc.sync.dma_start(out=sin_sb[:, s4 * TB:(s4 + 1) * TB],
                                  in_=sinT[:, s4 * TB:(s4 + 1) * TB])
            for h in range(NREP):
                nc.sync.dma_start(out=wo_sb[:, h, 0:TB * 2], in_=wo_r[:, h, 0:TB * 2])
                nc.sync.dma_start(out=wo_sb[:, h, TB * 2:D], in_=wo_r[:, h, TB * 2:D])
            for c in range(4):
                nc.sync.dma_start(out=x0[:, c, TB:2 * TB], in_=x_r[:, c, TB:2 * TB])

            def rope_start(src_ps, t0, nm):
                # rotate-half swap via a 213ns PE permutation matmul instead
                # of a 128KB SBUF->SBUF DMA (~10us latency in the chain).
                # raw is FR-typed (matmul-only consumer); the cos multiply
                # reads the PSUM source directly.
                raw = wpool.tile([P, TB], FR, tag="raw", name=f"raw{nm}")
                nc.scalar.copy(raw, src_ps)
                m1 = wpool.tile([P, TB], F32, tag="m1", name=f"m1{nm}")
                nc.vector.tensor_mul(m1, src_ps, cos_sb[:, t0:t0 + TB])
                return raw, m1, t0, nm

            def rope_finish(dst, st):
                raw, m1, t0, nm = st
                swp_ps = pss.tile([P, TB], F32, tag="big", bufs=7, name=f"swp{nm}")
                nc.tensor.matmul(swp_ps, swm_sb, raw, start=True, stop=True)
                m2 = wpool.tile([P, TB], F32, tag="m2", name=f"m2{nm}")
                nc.vector.tensor_mul(m2, swp_ps, sin_sb[:, t0:t0 + TB])
                nc.vector.tensor_add(dst, m1, m2)

            kps = {}

            def k_mm_jobs(bk):
                # per-c k-projection matmuls for block bk, as emission thunks
                # to be interleaved into the previous block's attention
                t0 = bk * TB
                xg, xoff = x_tiles[bk // 2], (bk % 2) * TB
                kps[bk] = pss.tile([P, TB], F32, tag="big", bufs=7, name=f"kps{bk}")

                def mk(c):
                    return lambda: nc.tensor.matmul(
                        kps[bk], wk_sb[:, c], xg[:, c, xoff:xoff + TB],
                        start=(c == 0), stop=(c == NC - 1))
                return [mk(c) for c in range(NC)]

            def wo_jobs(bk, oh_l, final):
                # one thunk per (ts, e) output tile of block bk's Wo partial
                t0 = bk * TB
                ysb = {}

                def mk(ts_, e):
                    def emit():
                        if e == 0:
                            ysb[ts_] = ypool.tile([P, D], BF16, tag="y",
                                                  name=f"y{bk}_{ts_}")
                        y_sb = ysb[ts_]
                        y_ps = pss.tile([P, TB], F32, tag="big", bufs=7,
                                        name=f"yps{bk}_{ts_}_{e}")
                        for h in range(NREP):
                            nc.tensor.matmul(y_ps, oh_l[h][:, ts_ * P:(ts_ + 1) * P],
                                             wo_sb[:, h, e * TB:(e + 1) * TB],
                                             start=(h == 0), stop=(h == NREP - 1))
                        if e % 2 == 0:
                            nc.scalar.copy(y_sb[:, e * TB:(e + 1) * TB], y_ps)
                        else:
                            nc.vector.tensor_copy(y_sb[:, e * TB:(e + 1) * TB], y_ps)
                        if final:
                            if ts_ == 3 and e >= 2:
                                for sp in range(2):
                                    nc.sync.dma_start(
                                        out=out[t0 + ts_ * P + sp * 64:
                                                t0 + ts_ * P + (sp + 1) * 64,
                                                e * TB:(e + 1) * TB],
                                        in_=y_sb[sp * 64:(sp + 1) * 64,
                                                 e * TB:(e + 1) * TB])
                            else:
                                nc.sync.dma_start(
                                    out=out[t0 + ts_ * P:t0 + (ts_ + 1) * P,
                                            e * TB:(e + 1) * TB],
                                    in_=y_sb[:, e * TB:(e + 1) * TB])
                        elif e % 2 == 1:
                            nc.sync.dma_start(
                                out=out[t0 + ts_ * P:t0 + (ts_ + 1) * P,
                                        (e - 1) * TB:(e + 1) * TB],
                                in_=y_sb[:, (e - 1) * TB:(e + 1) * TB])
                    return emit
                return [mk(ts_, e) for ts_ in range(4) for e in range(4)]

            def emit_proj_rest(bk, k_done):
                # v/q0 pass, transposes, q1-3 pass and all ropes for block bk.
                # If k_done, kps[bk] was already filled during the previous
                # block's attention; otherwise emit the k pass here first.
                t0 = bk * TB
                xg, xoff = x_tiles[bk // 2], (bk % 2) * TB
                if not k_done:
                    for job in k_mm_jobs(bk):
                        job()
                rk = rope_start(kps.pop(bk), t0, f"k{bk}")
                q_sb = [None] * NREP
                v_ps = pss.tile([P, TB], F32, tag="big", bufs=7, name=f"vps{bk}")
                q_ps = [None] * NREP
                q_ps[0] = pss.tile([P, TB], F32, tag="big", bufs=7, name=f"qps{bk}_0")
                for c in range(NC):
                    xt = xg[:, c, xoff:xoff + TB]
                    nc.tensor.matmul(v_ps, wv_sb[:, c], xt, start=(c == 0), stop=(c == NC - 1))
                    nc.tensor.matmul(q_ps[0], wq_sb[:, c, 0:P], xt,
                                     start=(c == 0), stop=(c == NC - 1))
                rope_finish(kT_sb[:, t0:t0 + TB], rk)
                r0 = rope_start(q_ps[0], t0, f"q{bk}_0")
                vtmp = wpool.tile([P, TB], F32, tag="vtmp", name=f"vtmp{bk}")
                nc.scalar.copy(vtmp, v_ps)
                for sj in range(4):
                    vt_ps = pss.tile([P, P], F32, tag="big", bufs=7, name=f"vt{bk}_{sj}")
                    nc.tensor.transpose(vt_ps, vtmp[:, sj * P:(sj + 1) * P], id_sb)
                    nc.vector.tensor_copy(v_sb[:, 4 * bk + sj], vt_ps)
                q_sb[0] = qpool.tile([P, TB], BF16, tag="qT", name=f"qT{bk}_0")
                rope_finish(q_sb[0], r0)
                for h in (1, 2, 3):
                    q_ps[h] = pss.tile([P, TB], F32, tag="big", bufs=7,
                                       name=f"qps{bk}_{h}")
                for c in range(NC):
                    for h in (1, 2, 3):
                        nc.tensor.matmul(q_ps[h], wq_sb[:, c, h * P:(h + 1) * P],
                                         xg[:, c, xoff:xoff + TB],
                                         start=(c == 0), stop=(c == NC - 1))
                rs = [rope_start(q_ps[h], t0, f"q{bk}_{h}") for h in (1, 2, 3)]
                for i, h in enumerate((1, 2, 3)):
                    q_sb[h] = qpool.tile([P, TB], BF16, tag="qT", name=f"qT{bk}_{h}")
                    rope_finish(q_sb[h], rs[i])
                return q_sb

            def emit_attention(bk, q_sb, extra):
                # attention for block bk; `extra` tensor-work thunks (previous
                # block's Wo tiles + next block's k projection) are drip-fed
                # between iterations so they run in the exp-gated gaps
                nj = 4 * bk + 4
                iters = NREP * nj
                done = 0
                it = 0
                oh_l = []
                for h in range(NREP):
                    cs_ps = pss.tile([1, TB], F32, tag="cs", bufs=1, name=f"cs{bk}_{h}")
                    ot_ps = pss.tile([P, TB], F32, tag="big", bufs=7, name=f"ot{bk}_{h}")

                    def csot(j, pt, js, stop):
                        nc.tensor.matmul(cs_ps[:, js:], ones_sb, pt[:, js:],
                                         start=(j == 0), stop=stop, skip_group_check=True)
                        nc.tensor.matmul(ot_ps[:, js:], v_sb[:, j], pt[:, js:],
                                         start=(j == 0), stop=stop, skip_group_check=True)

                    pend = None
                    for j in range(nj):
                        delta = j - 4 * bk
                        js = max(delta, 0) * P
                        s_ps = pss.tile([P, TB], F32, tag="big", bufs=7,
                                        name=f"s{bk}_{h}_{j}")
                        nc.tensor.matmul(s_ps[:, js:], kT_sb[:, j * P:(j + 1) * P],
                                         q_sb[h][:, js:], start=True, stop=True)
                        if delta >= 0:
                            nc.vector.tensor_add(s_ps[:, js:js + P], s_ps[:, js:js + P],
                                                 mtri_sb)
                        pt = ptpool.tile([P, TB], BF16, tag="pt", name=f"pt{bk}_{h}_{j}")
                        bidx = (h * NBLK + bk) * NJ + j
                        nc.scalar.activation(pt[:, js:], s_ps[:, js:], EXP,
                                             bias=cb_sb[:, bidx:bidx + 1])
                        if pend is not None:
                            csot(*pend, stop=False)
                        pend = (j, pt, js)
                        it += 1
                        # front-loaded drip: extras finish ~8 iters early so
                        # the oh-chain's vector/scalar ops aren't queued
                        # behind trailing Wo copies at block end
                        want = (it * len(extra)) // max(iters - 8, 1)
                        while done < min(want, len(extra)):
                            extra[done]()
                            done += 1
                    csot(*pend, stop=True)

                    rec = spool.tile([1, TB], F32, tag="rec", name=f"rec{bk}_{h}")
                    nc.vector.reciprocal_approx_fast(rec, cs_ps)
                    rbc = spool.tile([P, TB], F32, tag="rbc", name=f"rbc{bk}_{h}")
                    nc.gpsimd.partition_broadcast(rbc, rec)
                    oh = opool.tile([P, TB], BF16, tag="oh", name=f"oh{bk}_{h}")
                    nc.vector.tensor_mul(oh, ot_ps, rbc)
                    oh_l.append(oh)
                while done < len(extra):
                    extra[done]()
                    done += 1
                return oh_l

            # ---- pipelined schedule: attention(bk) absorbs wo(bk-1) and
            # the k projection of bk+1; wo(b3) is the only trailing phase ----
            q_sb = emit_proj_rest(0, k_done=False)
            prev_oh = None
            for bk in range(NBLK):
                extra = []
                if prev_oh is not None:
                    extra += wo_jobs(bk - 1, prev_oh, final=False)
                if bk + 1 < NBLK:
                    extra += k_mm_jobs(bk + 1)
                if bk == 0:
                    # first half of blocks 2-3's x: only 8 DMA engines get
                    # parked on long transfers, so block 1's rope swaps
                    # still find free engines
                    load_x_cols(1, 0, 8)
                prev_oh = emit_attention(bk, q_sb, extra)
                if bk + 1 < NBLK:
                    q_sb = emit_proj_rest(bk + 1, k_done=True)
                    if bk == 0:
                        # rest of pair 1, after block 1's swaps took engines;
                        # k(2) only reads c>=8 late in attention(1)
                        load_x_cols(1, 8, NC)
            for job in wo_jobs(NBLK - 1, prev_oh, final=True):
                job()

    nc.compile()
    return nc


def _prep_inputs(x, mask, freqs_cis, alibi_bias, Wq, Wk, Wv, Wo):
    """Host-side prep: partition-major packing, bf16 casts, RoPE tables,
    ALiBi bias decomposition."""
    f64 = np.float64
    idx = np.arange(HD)
    cos_full = freqs_cis[:, idx // 2]                     # [T, 128]
    sin_full = freqs_cis[:, (HD // 2) + idx // 2]         # [T, 128]
    sign = np.where(idx < HD // 2, -1.0, 1.0).astype(np.float32)
    cosT = np.ascontiguousarray(cos_full.T).astype(np.float32)          # [128, T]
    sinT = np.ascontiguousarray((sin_full * sign[None, :]).T).astype(np.float32)

    # triangle mask block: query i, key p -> 0 if i >= p else -1e9
    mtri = np.where(np.arange(P)[None, :] >= np.arange(P)[:, None],
                    0.0, -1e9).astype(np.float32)

    onesc = np.ones((P, 1), BF)
    idin = np.eye(P, dtype=np.float32)
    # rotate-half permutation as matmul lhsT: out[m] = in[(m+64)%128]
    swm = np.zeros((P, P), np.float32)
    swm[(np.arange(P) + P // 2) % P, np.arange(P)] = 1.0

    def pack(w, n):
        # [NC*P, n] -> [P, NC*n] partition-major
        return np.ascontiguousarray(
            w.reshape(-1, P, n).transpose(1, 0, 2).reshape(P, -1)).astype(BF)

    in_maps = []
    for c in range(8):
        b, gk = c // 4, c % 4
        slopes = np.array([-f64(alibi_bias[0, gk * NREP + hl, 1, 0]) for hl in range(NREP)])
        pvec = np.arange(P, dtype=f64)
        jvec = np.arange(NJ, dtype=f64)
        bkvec = np.arange(NBLK, dtype=f64)
        # cb[p, h, bk, j] = W*slope*(j*128 + p) - W*slope*(bk*512 + 511)
        cbv = (ALIBI_W * slopes[:, None, None, None]
               * (jvec[None, None, :, None] * P + pvec[None, None, None, :]
                  - (bkvec[None, :, None, None] * TB + (TB - 1))))
        cbm = np.ascontiguousarray(
            cbv.transpose(3, 0, 1, 2).reshape(P, NREP * NBLK * NJ)).astype(np.float32)
        in_maps.append({
            "xp": pack(np.ascontiguousarray(x[b].T), T),
            "wqp": pack(np.float32(SCALE) * Wq[:, gk * KVD:(gk + 1) * KVD], KVD),
            "wkp": pack(Wk[:, gk * P:(gk + 1) * P], P),
            "wvp": pack(Wv[:, gk * P:(gk + 1) * P], P),
            "wop": pack(Wo[gk * KVD:(gk + 1) * KVD, :], D),
            "cosT": cosT, "sinT": sinT,
            "cb": cbm, "mtri": mtri,
            "onesc": onesc, "idin": idin, "swm": swm,
        })
    return in_maps


def kernel(x, mask, freqs_cis, alibi_bias, Wq, Wk, Wv, Wo, _trace=False, _trace_kwargs=None):
    from concourse.bass_utils import run_bass_kernel_spmd

    if "nc" not in _cache:
        _cache["nc"] = _build()
    nc = _cache["nc"]

    in_maps = _prep_inputs(np.asarray(x, np.float32), np.asarray(mask, np.float32),
                           np.asarray(freqs_cis, np.float32), np.asarray(alibi_bias, np.float32),
                           np.asarray(Wq, np.float32), np.asarray(Wk, np.float32),
                           np.asarray(Wv, np.float32), np.asarray(Wo, np.float32))
    kw = {}
    if _trace:
        kw = dict(trace=True, **(_trace_kwargs or {}))
    res = run_bass_kernel_spmd(nc, in_maps, list(range(8)), **kw)

    full = np.zeros((B, T, D), np.float32)
    for c in range(8):
        full[c // 4] += np.asarray(res.results[c]["out"]).astype(np.float32)
    if _trace:
        _cache["last_trace"] = res
    return full


# revision 62
# speedup vs baseline: 1.0069x; 1.0069x over previous
"""GQA attention (RoPE + ALiBi + causal) on 8 trn2 NeuronCores.

Sharding: core c -> batch b = c//4, kv-group g = c%4 (4 q-heads + 1 kv-head
per core, column-sharded Wq/Wk/Wv, row-sharded Wo; host sums the 4 partial
Wo outputs per batch).

Optimizations vs the original fp32r version (638us -> ~316us):
- all matmul operands bf16 (host-cast; PSUM accumulate stays f32), bf16 out
  (norm rel err ~5e-3, well under the 2e-2 gate)
- host pre-packs every dram tensor partition-major so DMA packets are >=1KB
  contiguous per partition (256B-chunk weight loads were the startup gate)
- x loaded once per block-pair, resident in SBUF for all 6 projection matmuls
  per c-tile; first c-tiles split per-block so the first matmuls start sooner
- cross-block software pipeline: attention(bk), whose inner loop is exp-gated
  on the scalar engine, absorbs the previous block's Wo tiles and the next
  block's k projection as drip-fed tensor work (front-loaded so the softmax
  oh-chain isn't queued behind trailing copies); v/q0/q1-3 passes + ropes run
  between attentions, keeping PSUM <= 8 banks throughout
- attention inner loop software-pipelined (s[j+1..2] issued before cs/ot[j])
- causal diagonal tiles shortened: the dead below-block region is never
  computed (matmul/exp run on [:, delta*128:] only), and the causal mask add
  is a single [128,128] triangle block on the diagonal sub-tile
- softmax denominator via a ones-vector matmul accumulated in PSUM; 1/cs via
  reciprocal_approx_fast (plain vector reciprocal on a [1,512] AP is 3.3us)
- per-query ALiBi term dropped (softmax-invariant); per-key term + block
  offset ride the exp activation's per-partition bias, so no row-max pass
- RoPE rotate-half done as a 213ns f32r PE permutation matmul (exact 0/1
  matrix) instead of a 128KB SBUF->SBUF DMA whose ~10us transfer latency
  was exposed at block seams
- all DMA issued from the sync queue (gpsimd-issued DMA measured slower);
  final-block output writes split small so the tail drain is short
"""
import sys

if '/opt/trn_rl_repo' not in sys.path:
    sys.path.insert(0, '/opt/trn_rl_repo')

import numpy as np
import ml_dtypes

BF = ml_dtypes.bfloat16

B, T, D = 2, 2048, 2048
H, KV = 16, 4
HD = D // H          # 128
NREP = H // KV       # 4
KVD = 512            # per-core q width (4 heads x 128)
P = 128
TB = 512             # t-block
NBLK = T // TB       # 4
NC = D // P          # 16 contraction tiles
NJ = T // P          # 16 key tiles
ALIBI_W = 0.1
SCALE = (1.0 - ALIBI_W) / np.sqrt(np.float32(HD))

_cache = {}


def _build():
    from concourse import bacc, mybir
    from concourse.tile import TileContext

    F32 = mybir.dt.float32
    BF16 = mybir.dt.bfloat16
    FR = mybir.dt.float32r
    EXP = mybir.ActivationFunctionType.Exp

    nc = bacc.Bacc()
    xp = nc.declare_dram_parameter("xp", [P, NC * T], BF16, isOutput=False)
    wqp = nc.declare_dram_parameter("wqp", [P, NC * KVD], BF16, isOutput=False)
    wkp = nc.declare_dram_parameter("wkp", [P, NC * P], BF16, isOutput=False)
    wvp = nc.declare_dram_parameter("wvp", [P, NC * P], BF16, isOutput=False)
    wop = nc.declare_dram_parameter("wop", [P, NREP * D], BF16, isOutput=False)
    cosT = nc.declare_dram_parameter("cosT", [P, T], F32, isOutput=False)
    sinT = nc.declare_dram_parameter("sinT", [P, T], F32, isOutput=False)
    cb = nc.declare_dram_parameter("cb", [P, NREP * NBLK * NJ], F32, isOutput=False)
    mtri = nc.declare_dram_parameter("mtri", [P, P], F32, isOutput=False)
    onesc = nc.declare_dram_parameter("onesc", [P, 1], BF16, isOutput=False)
    idin = nc.declare_dram_parameter("idin", [P, P], F32, isOutput=False)
    swm = nc.declare_dram_parameter("swm", [P, P], F32, isOutput=False)
    out = nc.declare_dram_parameter("out", [T, D], BF16, isOutput=True)

    wq_r = wqp.rearrange("p (c n) -> p c n", n=KVD)
    wk_r = wkp.rearrange("p (c n) -> p c n", n=P)
    wv_r = wvp.rearrange("p (c n) -> p c n", n=P)
    wo_r = wop.rearrange("p (h e) -> p h e", e=D)
    x_r = xp.rearrange("p (c t) -> p c t", t=T)

    with TileContext(nc) as tc:
        with (
            tc.tile_pool(name="const", bufs=1) as cpool,
            tc.tile_pool(name="kv", bufs=1) as kvpool,
            tc.tile_pool(name="xin", bufs=2) as xpool,
            tc.tile_pool(name="work", bufs=2) as wpool,
            tc.tile_pool(name="qt", bufs=8) as qpool,
            tc.tile_pool(name="pt", bufs=4) as ptpool,
            tc.tile_pool(name="oh", bufs=8) as opool,
            tc.tile_pool(name="ysb", bufs=4) as ypool,
            tc.tile_pool(name="small", bufs=2) as spool,
            tc.tile_pool(name="ps", bufs=1, space="PSUM") as pss,
        ):
            # ---- resident constants (DMA order = need order) ----
            wk_sb = cpool.tile([P, NC, P], BF16)
            wv_sb = cpool.tile([P, NC, P], BF16)
            wq_sb = cpool.tile([P, NC, KVD], BF16)
            wo_sb = cpool.tile([P, NREP, D], BF16)
            cos_sb = cpool.tile([P, T], F32)
            sin_sb = cpool.tile([P, T], F32)
            cb_sb = cpool.tile([P, NREP * NBLK * NJ], F32)
            mtri_sb = cpool.tile([P, P], F32)
            ones_sb = cpool.tile([P, 1], BF16)
            id_sb = cpool.tile([P, P], F32)
            swm_sb = cpool.tile([P, P], FR)   # rotate-half permutation (lhsT)
            kT_sb = kvpool.tile([P, T], BF16)        # roped K, [d, s]
            v_sb = kvpool.tile([P, NJ, P], BF16)     # V tiles, [s, j, d']

            x_tiles = {}

            def load_x_cols(g, c_lo, c_hi):
                # full-width c-tiles keep 2KB DMA packets; issuing only half
                # the c-range at a time leaves DMA engines free for the
                # latency-critical rope-swap transfers
                if g not in x_tiles:
                    x_tiles[g] = xpool.tile([P, NC, 2 * TB], BF16, tag="x",
                                            name=f"x{g}")
                for c in range(c_lo, c_hi):
                    nc.sync.dma_start(out=x_tiles[g][:, c],
                                      in_=x_r[:, c, g * 2 * TB:(g + 1) * 2 * TB])

            # startup DMA order = need order: x c-tiles interleaved with the
            # weight chunks the first projection pass reads alongside them
            x0 = xpool.tile([P, NC, 2 * TB], BF16, tag="x", name="x0")
            x_tiles[0] = x0
            for c4 in range(4):
                for c in range(c4 * 4, c4 * 4 + 4):
                    if c4 == 0:
                        # first 4 c-tiles split per block: halves arrive ~2x
                        # sooner, so the very first matmuls start earlier
                        nc.sync.dma_start(out=x0[:, c, 0:TB], in_=x_r[:, c, 0:TB])
                    else:
                        nc.sync.dma_start(out=x0[:, c], in_=x_r[:, c, 0:2 * TB])
                nc.sync.dma_start(out=wk_sb[:, c4 * 4:(c4 + 1) * 4],
                                  in_=wk_r[:, c4 * 4:(c4 + 1) * 4])
                nc.sync.dma_start(out=wv_sb[:, c4 * 4:(c4 + 1) * 4],
                                  in_=wv_r[:, c4 * 4:(c4 + 1) * 4])
                nc.sync.dma_start(out=wq_sb[:, c4 * 4:(c4 + 1) * 4],
                                  in_=wq_r[:, c4 * 4:(c4 + 1) * 4])
            nc.sync.dma_start(out=cos_sb[:, 0:TB], in_=cosT[:, 0:TB])
            nc.sync.dma_start(out=sin_sb[:, 0:TB], in_=sinT[:, 0:TB])
            nc.sync.dma_start(out=cb_sb, in_=cb[:, :])
            nc.sync.dma_start(out=mtri_sb, in_=mtri[:, :])
            nc.sync.dma_start(out=ones_sb, in_=onesc[:, :])
            nc.sync.dma_start(out=id_sb, in_=idin[:, :])
            nc.sync.dma_start(out=swm_sb, in_=swm[:, :].bitcast(FR))
            for s4 in range(1, 4):
                nc.sync.dma_start(out=cos_sb[:, s4 * TB:(s4 + 1) * TB],
                                  in_=cosT[:, s4 * TB:(s4 + 1) * TB])
                nc.sync.dma_start(out=sin_sb[:, s4 * TB:(s4 + 1) * TB],
                                  in_=sinT[:, s4 * TB:(s4 + 1) * TB])
            for h in range(NREP):
                nc.sync.dma_start(out=wo_sb[:, h, 0:TB * 2], in_=wo_r[:, h, 0:TB * 2])
                nc.sync.dma_start(out=wo_sb[:, h, TB * 2:D], in_=wo_r[:, h, TB * 2:D])
            for c in range(4):
                nc.sync.dma_start(out=x0[:, c, TB:2 * TB], in_=x_r[:, c, TB:2 * TB])

            def rope_start(src_ps, t0, nm):
                # rotate-half swap via a 213ns PE permutation matmul instead
                # of a 128KB SBUF->SBUF DMA (~10us latency in the chain).
                # raw is FR-typed (matmul-only consumer); the cos multiply
                # reads the PSUM source directly.
                raw = wpool.tile([P, TB], FR, tag="raw", name=f"raw{nm}")
                nc.scalar.copy(raw, src_ps)
                m1 = wpool.tile([P, TB], F32, tag="m1", name=f"m1{nm}")
                nc.vector.tensor_mul(m1, src_ps, cos_sb[:, t0:t0 + TB])
                return raw, m1, t0, nm

            def rope_finish(dst, st):
                raw, m1, t0, nm = st
                swp_ps = pss.tile([P, TB], F32, tag="big", bufs=7, name=f"swp{nm}")
                nc.tensor.matmul(swp_ps, swm_sb, raw, start=True, stop=True)
                m2 = wpool.tile([P, TB], F32, tag="m2", name=f"m2{nm}")
                nc.vector.tensor_mul(m2, swp_ps, sin_sb[:, t0:t0 + TB])
                nc.vector.tensor_add(dst, m1, m2)

            kps = {}

            def k_mm_jobs(bk):
                # per-c k-projection matmuls for block bk, as emission thunks
                # to be interleaved into the previous block's attention
                t0 = bk * TB
                xg, xoff = x_tiles[bk // 2], (bk % 2) * TB
                kps[bk] = pss.tile([P, TB], F32, tag="big", bufs=7, name=f"kps{bk}")

                def mk(c):
                    return lambda: nc.tensor.matmul(
                        kps[bk], wk_sb[:, c], xg[:, c, xoff:xoff + TB],
                        start=(c == 0), stop=(c == NC - 1))
                return [mk(c) for c in range(NC)]

            def wo_jobs(bk, oh_l, final):
                # one thunk per (ts, e) output tile of block bk's Wo partial
                t0 = bk * TB
                ysb = {}

                def mk(ts_, e):
                    def emit():
                        if e == 0:
                            ysb[ts_] = ypool.tile([P, D], BF16, tag="y",
                                                  name=f"y{bk}_{ts_}")
                        y_sb = ysb[ts_]
                        y_ps = pss.tile([P, TB], F32, tag="big", bufs=7,
                                        name=f"yps{bk}_{ts_}_{e}")
                        for h in range(NREP):
                            nc.tensor.matmul(y_ps, oh_l[h][:, ts_ * P:(ts_ + 1) * P],
                                             wo_sb[:, h, e * TB:(e + 1) * TB],
                                             start=(h == 0), stop=(h == NREP - 1))
                        if e % 2 == 0:
                            nc.scalar.copy(y_sb[:, e * TB:(e + 1) * TB], y_ps)
                        else:
                            nc.vector.tensor_copy(y_sb[:, e * TB:(e + 1) * TB], y_ps)
                        if final:
                            if ts_ == 3 and e >= 2:
                                for sp in range(2):
                                    nc.sync.dma_start(
                                        out=out[t0 + ts_ * P + sp * 64:
                                                t0 + ts_ * P + (sp + 1) * 64,
                                                e * TB:(e + 1) * TB],
                                        in_=y_sb[sp * 64:(sp + 1) * 64,
                                                 e * TB:(e + 1) * TB])
                            else:
                                nc.sync.dma_start(
                                    out=out[t0 + ts_ * P:t0 + (ts_ + 1) * P,
                                            e * TB:(e + 1) * TB],
                                    in_=y_sb[:, e * TB:(e + 1) * TB])
                        elif e % 2 == 1:
                            nc.sync.dma_start(
                                out=out[t0 + ts_ * P:t0 + (ts_ + 1) * P,
                                        (e - 1) * TB:(e + 1) * TB],
                                in_=y_sb[:, (e - 1) * TB:(e + 1) * TB])
                    return emit
                return [mk(ts_, e) for ts_ in range(4) for e in range(4)]

            def emit_proj_rest(bk, k_done):
                # v/q0 pass, transposes, q1-3 pass and all ropes for block bk.
                # If k_done, kps[bk] was already filled during the previous
                # block's attention; otherwise emit the k pass here first.
                t0 = bk * TB
                xg, xoff = x_tiles[bk // 2], (bk % 2) * TB
                if not k_done:
                    for job in k_mm_jobs(bk):
                        job()
                rk = rope_start(kps.pop(bk), t0, f"k{bk}")
                q_sb = [None] * NREP
                v_ps = pss.tile([P, TB], F32, tag="big", bufs=7, name=f"vps{bk}")
                q_ps = [None] * NREP
                q_ps[0] = pss.tile([P, TB], F32, tag="big", bufs=7, name=f"qps{bk}_0")
                for c in range(NC):
                    xt = xg[:, c, xoff:xoff + TB]
                    nc.tensor.matmul(v_ps, wv_sb[:, c], xt, start=(c == 0), stop=(c == NC - 1))
                    nc.tensor.matmul(q_ps[0], wq_sb[:, c, 0:P], xt,
                                     start=(c == 0), stop=(c == NC - 1))
                rope_finish(kT_sb[:, t0:t0 + TB], rk)
                r0 = rope_start(q_ps[0], t0, f"q{bk}_0")
                vtmp = wpool.tile([P, TB], F32, tag="vtmp", name=f"vtmp{bk}")
                nc.scalar.copy(vtmp, v_ps)
                for sj in range(4):
                    vt_ps = pss.tile([P, P], F32, tag="big", bufs=7, name=f"vt{bk}_{sj}")
                    nc.tensor.transpose(vt_ps, vtmp[:, sj * P:(sj + 1) * P], id_sb)
                    nc.vector.tensor_copy(v_sb[:, 4 * bk + sj], vt_ps)
                q_sb[0] = qpool.tile([P, TB], BF16, tag="qT", name=f"qT{bk}_0")
                rope_finish(q_sb[0], r0)
                for h in (1, 2, 3):
                    q_ps[h] = pss.tile([P, TB], F32, tag="big", bufs=7,
                                       name=f"qps{bk}_{h}")
                for c in range(NC):
                    for h in (1, 2, 3):
                        nc.tensor.matmul(q_ps[h], wq_sb[:, c, h * P:(h + 1) * P],
                                         xg[:, c, xoff:xoff + TB],
                                         start=(c == 0), stop=(c == NC - 1))
                rs = [rope_start(q_ps[h], t0, f"q{bk}_{h}") for h in (1, 2, 3)]
                for i, h in enumerate((1, 2, 3)):
                    q_sb[h] = qpool.tile([P, TB], BF16, tag="qT", name=f"qT{bk}_{h}")
                    rope_finish(q_sb[h], rs[i])
                return q_sb

            def emit_attention(bk, q_sb, extra):
                # attention for block bk; `extra` tensor-work thunks (previous
                # block's Wo tiles + next block's k projection) are drip-fed
                # between iterations so they run in the exp-gated gaps
                nj = 4 * bk + 4
                iters = NREP * nj
                done = 0
                it = 0
                oh_l = []
                for h in range(NREP):
                    cs_ps = pss.tile([1, TB], F32, tag="cs", bufs=1, name=f"cs{bk}_{h}")
                    ot_ps = pss.tile([P, TB], F32, tag="big", bufs=7, name=f"ot{bk}_{h}")

                    def csot(j, pt, js, stop):
                        nc.tensor.matmul(cs_ps[:, js:], ones_sb, pt[:, js:],
                                         start=(j == 0), stop=stop, skip_group_check=True)
                        nc.tensor.matmul(ot_ps[:, js:], v_sb[:, j], pt[:, js:],
                                         start=(j == 0), stop=stop, skip_group_check=True)

                    pend = None
                    for j in range(nj):
                        delta = j - 4 * bk
                        js = max(delta, 0) * P
                        s_ps = pss.tile([P, TB], F32, tag="big", bufs=7,
                                        name=f"s{bk}_{h}_{j}")
                        nc.tensor.matmul(s_ps[:, js:], kT_sb[:, j * P:(j + 1) * P],
                                         q_sb[h][:, js:], start=True, stop=True)
                        if delta >= 0:
                            nc.vector.tensor_add(s_ps[:, js:js + P], s_ps[:, js:js + P],
                                                 mtri_sb)
                        pt = ptpool.tile([P, TB], BF16, tag="pt", name=f"pt{bk}_{h}_{j}")
                        bidx = (h * NBLK + bk) * NJ + j
                        nc.scalar.activation(pt[:, js:], s_ps[:, js:], EXP,
                                             bias=cb_sb[:, bidx:bidx + 1])
                        if pend is not None:
                            csot(*pend, stop=False)
                        pend = (j, pt, js)
                        it += 1
                        # front-loaded drip: extras finish ~8 iters early so
                        # the oh-chain's vector/scalar ops aren't queued
                        # behind trailing Wo copies at block end
                        want = (it * len(extra)) // max(iters - 8, 1)
                        while done < min(want, len(extra)):
                            extra[done]()
                            done += 1
                    csot(*pend, stop=True)

                    rec = spool.tile([1, TB], F32, tag="rec", name=f"rec{bk}_{h}")
                    nc.vector.reciprocal_approx_fast(rec, cs_ps)
                    rbc = spool.tile([P, TB], F32, tag="rbc", name=f"rbc{bk}_{h}")
                    nc.gpsimd.partition_broadcast(rbc, rec)
                    oh = opool.tile([P, TB], BF16, tag="oh", name=f"oh{bk}_{h}")
                    nc.vector.tensor_mul(oh, ot_ps, rbc)
                    oh_l.append(oh)
                while done < len(extra):
                    extra[done]()
                    done += 1
                return oh_l

            # ---- pipelined schedule: attention(bk) absorbs wo(bk-1) and
            # the k projection of bk+1; wo(b3) is the only trailing phase ----
            q_sb = emit_proj_rest(0, k_done=False)
            prev_oh = None
            for bk in range(NBLK):
                extra = []
                if prev_oh is not None:
                    extra += wo_jobs(bk - 1, prev_oh, final=False)
                if bk + 1 < NBLK:
                    extra += k_mm_jobs(bk + 1)
                if bk == 0:
                    # first half of blocks 2-3's x: only 8 DMA engines get
                    # parked on long transfers, so block 1's rope swaps
                    # still find free engines
                    load_x_cols(1, 0, 8)
                prev_oh = emit_attention(bk, q_sb, extra)
                if bk + 1 < NBLK:
                    q_sb = emit_proj_rest(bk + 1, k_done=True)
                    if bk == 0:
                        # rest of pair 1, after block 1's swaps took engines;
                        # k(2) only reads c>=8 late in attention(1)
                        load_x_cols(1, 8, NC)
            for job in wo_jobs(NBLK - 1, prev_oh, final=True):
                job()

    nc.compile()
    return nc


def _prep_inputs(x, mask, freqs_cis, alibi_bias, Wq, Wk, Wv, Wo):
    """Host-side prep: partition-major packing, bf16 casts, RoPE tables,
    ALiBi bias decomposition."""
    f64 = np.float64
    idx = np.arange(HD)
    cos_full = freqs_cis[:, idx // 2]                     # [T, 128]
    sin_full = freqs_cis[:, (HD // 2) + idx // 2]         # [T, 128]
    sign = np.where(idx < HD // 2, -1.0, 1.0).astype(np.float32)
    cosT = np.ascontiguousarray(cos_full.T).astype(np.float32)          # [128, T]
    sinT = np.ascontiguousarray((sin_full * sign[None, :]).T).astype(np.float32)

    # triangle mask block: query i, key p -> 0 if i >= p else -1e9
    mtri = np.where(np.arange(P)[None, :] >= np.arange(P)[:, None],
                    0.0, -1e9).astype(np.float32)

    onesc = np.ones((P, 1), BF)
    idin = np.eye(P, dtype=np.float32)
    # rotate-half permutation as matmul lhsT: out[m] = in[(m+64)%128]
    swm = np.zeros((P, P), np.float32)
    swm[(np.arange(P) + P // 2) % P, np.arange(P)] = 1.0

    def pack(w, n):
        # [NC*P, n] -> [P, NC*n] partition-major
        return np.ascontiguousarray(
            w.reshape(-1, P, n).transpose(1, 0, 2).reshape(P, -1)).astype(BF)

    in_maps = []
    for c in range(8):
        b, gk = c // 4, c % 4
        slopes = np.array([-f64(alibi_bias[0, gk * NREP + hl, 1, 0]) for hl in range(NREP)])
        pvec = np.arange(P, dtype=f64)
        jvec = np.arange(NJ, dtype=f64)
        bkvec = np.arange(NBLK, dtype=f64)
        # cb[p, h, bk, j] = W*slope*(j*128 + p) - W*slope*(bk*512 + 511)
        cbv = (ALIBI_W * slopes[:, None, None, None]
               * (jvec[None, None, :, None] * P + pvec[None, None, None, :]
                  - (bkvec[None, :, None, None] * TB + (TB - 1))))
        cbm = np.ascontiguousarray(
            cbv.transpose(3, 0, 1, 2).reshape(P, NREP * NBLK * NJ)).astype(np.float32)
        in_maps.append({
            "xp": pack(np.ascontiguousarray(x[b].T), T),
            "wqp": pack(np.float32(SCALE) * Wq[:, gk * KVD:(gk + 1) * KVD], KVD),
            "wkp": pack(Wk[:, gk * P:(gk + 1) * P], P),
            "wvp": pack(Wv[:, gk * P:(gk + 1) * P], P),
            "wop": pack(Wo[gk * KVD:(gk + 1) * KVD, :], D),
            "cosT": cosT, "sinT": sinT,
            "cb": cbm, "mtri": mtri,
            "onesc": onesc, "idin": idin, "swm": swm,
        })
    return in_maps


def kernel(x, mask, freqs_cis, alibi_bias, Wq, Wk, Wv, Wo, _trace=False, _trace_kwargs=None):
    from concourse.bass_utils import run_bass_kernel_spmd

    if "nc" not in _cache:
        _cache["nc"] = _build()
    nc = _cache["nc"]

    in_maps = _prep_inputs(np.asarray(x, np.float32), np.asarray(mask, np.float32),
                           np.asarray(freqs_cis, np.float32), np.asarray(alibi_bias, np.float32),
                           np.asarray(Wq, np.float32), np.asarray(Wk, np.float32),
                           np.asarray(Wv, np.float32), np.asarray(Wo, np.float32))
    kw = {}
    if _trace:
        kw = dict(trace=True, **(_trace_kwargs or {}))
    res = run_bass_kernel_spmd(nc, in_maps, list(range(8)), **kw)

    full = np.zeros((B, T, D), np.float32)
    for c in range(8):
        full[c // 4] += np.asarray(res.results[c]["out"]).astype(np.float32)
    if _trace:
        _cache["last_trace"] = res
    return full


# revision 64
# speedup vs baseline: 1.0138x; 1.0069x over previous
"""GQA attention (RoPE + ALiBi + causal) on 8 trn2 NeuronCores.

Sharding: core c -> batch b = c//4, kv-group g = c%4 (4 q-heads + 1 kv-head
per core, column-sharded Wq/Wk/Wv, row-sharded Wo; host sums the 4 partial
Wo outputs per batch).

Optimizations vs the original fp32r version (638us -> ~316us):
- all matmul operands bf16 (host-cast; PSUM accumulate stays f32), bf16 out
  (norm rel err ~5e-3, well under the 2e-2 gate)
- host pre-packs every dram tensor partition-major so DMA packets are >=1KB
  contiguous per partition (256B-chunk weight loads were the startup gate)
- x loaded once per block-pair, resident in SBUF for all 6 projection matmuls
  per c-tile; first c-tiles split per-block so the first matmuls start sooner
- cross-block software pipeline: attention(bk), whose inner loop is exp-gated
  on the scalar engine, absorbs the previous block's Wo tiles and the next
  block's k projection as drip-fed tensor work (front-loaded so the softmax
  oh-chain isn't queued behind trailing copies); v/q0/q1-3 passes + ropes run
  between attentions, keeping PSUM <= 8 banks throughout
- attention inner loop software-pipelined (s[j+1..2] issued before cs/ot[j])
- causal diagonal tiles shortened: the dead below-block region is never
  computed (matmul/exp run on [:, delta*128:] only), and the causal mask add
  is a single [128,128] triangle block on the diagonal sub-tile
- softmax denominator via a ones-vector matmul accumulated in PSUM; 1/cs via
  reciprocal_approx_fast (plain vector reciprocal on a [1,512] AP is 3.3us)
- per-query ALiBi term dropped (softmax-invariant); per-key term + block
  offset ride the exp activation's per-partition bias, so no row-max pass
- RoPE rotate-half done as a 213ns f32r PE permutation matmul (exact 0/1
  matrix) instead of a 128KB SBUF->SBUF DMA whose ~10us transfer latency
  was exposed at block seams
- all DMA issued from the sync queue (gpsimd-issued DMA measured slower);
  final-block output writes split small so the tail drain is short
"""
import sys

if '/opt/trn_rl_repo' not in sys.path:
    sys.path.insert(0, '/opt/trn_rl_repo')

import numpy as np
import ml_dtypes

BF = ml_dtypes.bfloat16

B, T, D = 2, 2048, 2048
H, KV = 16, 4
HD = D // H          # 128
NREP = H // KV       # 4
KVD = 512            # per-core q width (4 heads x 128)
P = 128
TB = 512             # t-block
NBLK = T // TB       # 4
NC = D // P          # 16 contraction tiles
NJ = T // P          # 16 key tiles
ALIBI_W = 0.1
SCALE = (1.0 - ALIBI_W) / np.sqrt(np.float32(HD))

_cache = {}


def _build():
    from concourse import bacc, mybir
    from concourse.tile import TileContext

    F32 = mybir.dt.float32
    BF16 = mybir.dt.bfloat16
    FR = mybir.dt.float32r
    EXP = mybir.ActivationFunctionType.Exp

    nc = bacc.Bacc()
    xp = nc.declare_dram_parameter("xp", [P, NC * T], BF16, isOutput=False)
    wqp = nc.declare_dram_parameter("wqp", [P, NC * KVD], BF16, isOutput=False)
    wkp = nc.declare_dram_parameter("wkp", [P, NC * P], BF16, isOutput=False)
    wvp = nc.declare_dram_parameter("wvp", [P, NC * P], BF16, isOutput=False)
    wop = nc.declare_dram_parameter("wop", [P, NREP * D], BF16, isOutput=False)
    cosT = nc.declare_dram_parameter("cosT", [P, T], F32, isOutput=False)
    sinT = nc.declare_dram_parameter("sinT", [P, T], F32, isOutput=False)
    cb = nc.declare_dram_parameter("cb", [P, NREP * NBLK * NJ], F32, isOutput=False)
    mtri = nc.declare_dram_parameter("mtri", [P, P], F32, isOutput=False)
    onesc = nc.declare_dram_parameter("onesc", [P, 1], BF16, isOutput=False)
    idin = nc.declare_dram_parameter("idin", [P, P], F32, isOutput=False)
    swm = nc.declare_dram_parameter("swm", [P, P], F32, isOutput=False)
    out = nc.declare_dram_parameter("out", [T, D], BF16, isOutput=True)

    wq_r = wqp.rearrange("p (c n) -> p c n", n=KVD)
    wk_r = wkp.rearrange("p (c n) -> p c n", n=P)
    wv_r = wvp.rearrange("p (c n) -> p c n", n=P)
    wo_r = wop.rearrange("p (h e) -> p h e", e=D)
    x_r = xp.rearrange("p (c t) -> p c t", t=T)

    with TileContext(nc) as tc:
        with (
            tc.tile_pool(name="const", bufs=1) as cpool,
            tc.tile_pool(name="kv", bufs=1) as kvpool,
            tc.tile_pool(name="xin", bufs=2) as xpool,
            tc.tile_pool(name="work", bufs=2) as wpool,
            tc.tile_pool(name="qt", bufs=8) as qpool,
            tc.tile_pool(name="pt", bufs=4) as ptpool,
            tc.tile_pool(name="oh", bufs=8) as opool,
            tc.tile_pool(name="ysb", bufs=4) as ypool,
            tc.tile_pool(name="small", bufs=2) as spool,
            tc.tile_pool(name="ps", bufs=1, space="PSUM") as pss,
        ):
            # ---- resident constants (DMA order = need order) ----
            wk_sb = cpool.tile([P, NC, P], BF16)
            wv_sb = cpool.tile([P, NC, P], BF16)
            wq_sb = cpool.tile([P, NC, KVD], BF16)
            wo_sb = cpool.tile([P, NREP, D], BF16)
            cos_sb = cpool.tile([P, T], F32)
            sin_sb = cpool.tile([P, T], F32)
            cb_sb = cpool.tile([P, NREP * NBLK * NJ], F32)
            mtri_sb = cpool.tile([P, P], F32)
            ones_sb = cpool.tile([P, 1], BF16)
            id_sb = cpool.tile([P, P], F32)
            swm_sb = cpool.tile([P, P], FR)   # rotate-half permutation (lhsT)
            kT_sb = kvpool.tile([P, T], BF16)        # roped K, [d, s]
            v_sb = kvpool.tile([P, NJ, P], BF16)     # V tiles, [s, j, d']

            x_tiles = {}

            def load_x_cols(g, c_lo, c_hi):
                # full-width c-tiles keep 2KB DMA packets; issuing only half
                # the c-range at a time leaves DMA engines free for the
                # latency-critical rope-swap transfers
                if g not in x_tiles:
                    x_tiles[g] = xpool.tile([P, NC, 2 * TB], BF16, tag="x",
                                            name=f"x{g}")
                for c in range(c_lo, c_hi):
                    nc.sync.dma_start(out=x_tiles[g][:, c],
                                      in_=x_r[:, c, g * 2 * TB:(g + 1) * 2 * TB])

            # startup DMA order = need order: x c-tiles interleaved with the
            # weight chunks the first projection pass reads alongside them
            x0 = xpool.tile([P, NC, 2 * TB], BF16, tag="x", name="x0")
            x_tiles[0] = x0
            for c4 in range(4):
                for c in range(c4 * 4, c4 * 4 + 4):
                    if c4 == 0:
                        # first 4 c-tiles split per block: halves arrive ~2x
                        # sooner, so the very first matmuls start earlier
                        nc.sync.dma_start(out=x0[:, c, 0:TB], in_=x_r[:, c, 0:TB])
                    else:
                        nc.sync.dma_start(out=x0[:, c], in_=x_r[:, c, 0:2 * TB])
                nc.sync.dma_start(out=wk_sb[:, c4 * 4:(c4 + 1) * 4],
                                  in_=wk_r[:, c4 * 4:(c4 + 1) * 4])
                nc.sync.dma_start(out=wv_sb[:, c4 * 4:(c4 + 1) * 4],
                                  in_=wv_r[:, c4 * 4:(c4 + 1) * 4])
                nc.sync.dma_start(out=wq_sb[:, c4 * 4:(c4 + 1) * 4],
                                  in_=wq_r[:, c4 * 4:(c4 + 1) * 4])
            nc.sync.dma_start(out=cos_sb[:, 0:TB], in_=cosT[:, 0:TB])
            nc.sync.dma_start(out=sin_sb[:, 0:TB], in_=sinT[:, 0:TB])
            nc.sync.dma_start(out=cb_sb, in_=cb[:, :])
            nc.sync.dma_start(out=mtri_sb, in_=mtri[:, :])
            nc.sync.dma_start(out=ones_sb, in_=onesc[:, :])
            nc.sync.dma_start(out=id_sb, in_=idin[:, :])
            nc.sync.dma_start(out=swm_sb, in_=swm[:, :].bitcast(FR))
            for s4 in range(1, 4):
                nc.sync.dma_start(out=cos_sb[:, s4 * TB:(s4 + 1) * TB],
                                  in_=cosT[:, s4 * TB:(s4 + 1) * TB])
                nc.sync.dma_start(out=sin_sb[:, s4 * TB:(s4 + 1) * TB],
                                  in_=sinT[:, s4 * TB:(s4 + 1) * TB])
            for h in range(NREP):
                nc.sync.dma_start(out=wo_sb[:, h, 0:TB * 2], in_=wo_r[:, h, 0:TB * 2])
                nc.sync.dma_start(out=wo_sb[:, h, TB * 2:D], in_=wo_r[:, h, TB * 2:D])
            for c in range(4):
                nc.sync.dma_start(out=x0[:, c, TB:2 * TB], in_=x_r[:, c, TB:2 * TB])

            def rope_start(src_ps, t0, nm):
                # rotate-half swap via a 213ns PE permutation matmul instead
                # of a 128KB SBUF->SBUF DMA (~10us latency in the chain).
                # raw is FR-typed (matmul-only consumer); the cos multiply
                # reads the PSUM source directly.
                raw = wpool.tile([P, TB], FR, tag="raw", name=f"raw{nm}")
                nc.scalar.copy(raw, src_ps)
                m1 = wpool.tile([P, TB], F32, tag="m1", name=f"m1{nm}")
                nc.vector.tensor_mul(m1, src_ps, cos_sb[:, t0:t0 + TB])
                return raw, m1, t0, nm

            def rope_finish(dst, st):
                raw, m1, t0, nm = st
                swp_ps = pss.tile([P, TB], F32, tag="big", bufs=7, name=f"swp{nm}")
                nc.tensor.matmul(swp_ps, swm_sb, raw, start=True, stop=True)
                m2 = wpool.tile([P, TB], F32, tag="m2", name=f"m2{nm}")
                nc.vector.tensor_mul(m2, swp_ps, sin_sb[:, t0:t0 + TB])
                nc.vector.tensor_add(dst, m1, m2)

            kps = {}

            def k_mm_jobs(bk):
                # per-c k-projection matmuls for block bk, as emission thunks
                # to be interleaved into the previous block's attention
                t0 = bk * TB
                xg, xoff = x_tiles[bk // 2], (bk % 2) * TB
                kps[bk] = pss.tile([P, TB], F32, tag="big", bufs=7, name=f"kps{bk}")

                def mk(c):
                    return lambda: nc.tensor.matmul(
                        kps[bk], wk_sb[:, c], xg[:, c, xoff:xoff + TB],
                        start=(c == 0), stop=(c == NC - 1))
                return [mk(c) for c in range(NC)]

            def wo_jobs(bk, oh_l, final):
                # one thunk per (ts, e) output tile of block bk's Wo partial
                t0 = bk * TB
                ysb = {}

                def mk(ts_, e):
                    def emit():
                        if e == 0:
                            ysb[ts_] = ypool.tile([P, D], BF16, tag="y",
                                                  name=f"y{bk}_{ts_}")
                        y_sb = ysb[ts_]
                        y_ps = pss.tile([P, TB], F32, tag="big", bufs=7,
                                        name=f"yps{bk}_{ts_}_{e}")
                        for h in range(NREP):
                            nc.tensor.matmul(y_ps, oh_l[h][:, ts_ * P:(ts_ + 1) * P],
                                             wo_sb[:, h, e * TB:(e + 1) * TB],
                                             start=(h == 0), stop=(h == NREP - 1))
                        if e % 2 == 0:
                            nc.scalar.copy(y_sb[:, e * TB:(e + 1) * TB], y_ps)
                        else:
                            nc.vector.tensor_copy(y_sb[:, e * TB:(e + 1) * TB], y_ps)
                        if final:
                            if ts_ == 3 and e >= 2:
                                for sp in range(2):
                                    nc.sync.dma_start(
                                        out=out[t0 + ts_ * P + sp * 64:
                                                t0 + ts_ * P + (sp + 1) * 64,
                                                e * TB:(e + 1) * TB],
                                        in_=y_sb[sp * 64:(sp + 1) * 64,
                                                 e * TB:(e + 1) * TB])
                            else:
                                nc.sync.dma_start(
                                    out=out[t0 + ts_ * P:t0 + (ts_ + 1) * P,
                                            e * TB:(e + 1) * TB],
                                    in_=y_sb[:, e * TB:(e + 1) * TB])
                        elif e % 2 == 1:
                            nc.sync.dma_start(
                                out=out[t0 + ts_ * P:t0 + (ts_ + 1) * P,
                                        (e - 1) * TB:(e + 1) * TB],
                                in_=y_sb[:, (e - 1) * TB:(e + 1) * TB])
                    return emit
                return [mk(ts_, e) for ts_ in range(4) for e in range(4)]

            def emit_proj_rest(bk, k_done):
                # v/q0 pass, transposes, q1-3 pass and all ropes for block bk.
                # If k_done, kps[bk] was already filled during the previous
                # block's attention; otherwise emit the k pass here first.
                t0 = bk * TB
                xg, xoff = x_tiles[bk // 2], (bk % 2) * TB
                if not k_done:
                    for job in k_mm_jobs(bk):
                        job()
                rk = rope_start(kps.pop(bk), t0, f"k{bk}")
                q_sb = [None] * NREP
                v_ps = pss.tile([P, TB], F32, tag="big", bufs=7, name=f"vps{bk}")
                q_ps = [None] * NREP
                q_ps[0] = pss.tile([P, TB], F32, tag="big", bufs=7, name=f"qps{bk}_0")
                for c in range(NC):
                    xt = xg[:, c, xoff:xoff + TB]
                    nc.tensor.matmul(v_ps, wv_sb[:, c], xt, start=(c == 0), stop=(c == NC - 1))
                    nc.tensor.matmul(q_ps[0], wq_sb[:, c, 0:P], xt,
                                     start=(c == 0), stop=(c == NC - 1))
                rope_finish(kT_sb[:, t0:t0 + TB], rk)
                r0 = rope_start(q_ps[0], t0, f"q{bk}_0")
                vtmp = wpool.tile([P, TB], F32, tag="vtmp", name=f"vtmp{bk}")
                nc.scalar.copy(vtmp, v_ps)
                for sj in range(4):
                    vt_ps = pss.tile([P, P], F32, tag="big", bufs=7, name=f"vt{bk}_{sj}")
                    nc.tensor.transpose(vt_ps, vtmp[:, sj * P:(sj + 1) * P], id_sb)
                    nc.vector.tensor_copy(v_sb[:, 4 * bk + sj], vt_ps)
                q_sb[0] = qpool.tile([P, TB], BF16, tag="qT", name=f"qT{bk}_0")
                rope_finish(q_sb[0], r0)
                for h in (1, 2, 3):
                    q_ps[h] = pss.tile([P, TB], F32, tag="big", bufs=7,
                                       name=f"qps{bk}_{h}")
                for c in range(NC):
                    for h in (1, 2, 3):
                        nc.tensor.matmul(q_ps[h], wq_sb[:, c, h * P:(h + 1) * P],
                                         xg[:, c, xoff:xoff + TB],
                                         start=(c == 0), stop=(c == NC - 1))
                rs = [rope_start(q_ps[h], t0, f"q{bk}_{h}") for h in (1, 2, 3)]
                for i, h in enumerate((1, 2, 3)):
                    q_sb[h] = qpool.tile([P, TB], BF16, tag="qT", name=f"qT{bk}_{h}")
                    rope_finish(q_sb[h], rs[i])
                return q_sb

            def emit_attention(bk, q_sb, extra):
                # attention for block bk; `extra` tensor-work thunks (previous
                # block's Wo tiles + next block's k projection) are drip-fed
                # between iterations so they run in the exp-gated gaps
                nj = 4 * bk + 4
                iters = NREP * nj
                done = 0
                it = 0
                oh_l = []
                for h in range(NREP):
                    cs_ps = pss.tile([1, TB], F32, tag="cs", bufs=1, name=f"cs{bk}_{h}")
                    ot_ps = pss.tile([P, TB], F32, tag="big", bufs=7, name=f"ot{bk}_{h}")

                    def csot(j, pt, js, stop):
                        nc.tensor.matmul(cs_ps[:, js:], ones_sb, pt[:, js:],
                                         start=(j == 0), stop=stop, skip_group_check=True)
                        nc.tensor.matmul(ot_ps[:, js:], v_sb[:, j], pt[:, js:],
                                         start=(j == 0), stop=stop, skip_group_check=True)

                    pend = None
                    for j in range(nj):
                        delta = j - 4 * bk
                        js = max(delta, 0) * P
                        s_ps = pss.tile([P, TB], F32, tag="big", bufs=7,
                                        name=f"s{bk}_{h}_{j}")
                        nc.tensor.matmul(s_ps[:, js:], kT_sb[:, j * P:(j + 1) * P],
                                         q_sb[h][:, js:], start=True, stop=True)
                        if delta >= 0:
                            nc.vector.tensor_add(s_ps[:, js:js + P], s_ps[:, js:js + P],
                                                 mtri_sb)
                        pt = ptpool.tile([P, TB], BF16, tag="pt", name=f"pt{bk}_{h}_{j}")
                        bidx = (h * NBLK + bk) * NJ + j
                        nc.scalar.activation(pt[:, js:], s_ps[:, js:], EXP,
                                             bias=cb_sb[:, bidx:bidx + 1])
                        if pend is not None:
                            csot(*pend, stop=False)
                        pend = (j, pt, js)
                        it += 1
                        # front-loaded drip: extras finish ~8 iters early so
                        # the oh-chain's vector/scalar ops aren't queued
                        # behind trailing Wo copies at block end
                        want = (it * len(extra)) // max(iters - 8, 1)
                        while done < min(want, len(extra)):
                            extra[done]()
                            done += 1
                    csot(*pend, stop=True)

                    rec = spool.tile([1, TB], F32, tag="rec", name=f"rec{bk}_{h}")
                    nc.vector.reciprocal_approx_fast(rec, cs_ps)
                    rbc = spool.tile([P, TB], F32, tag="rbc", name=f"rbc{bk}_{h}")
                    nc.gpsimd.partition_broadcast(rbc, rec)
                    oh = opool.tile([P, TB], BF16, tag="oh", name=f"oh{bk}_{h}")
                    nc.vector.tensor_mul(oh, ot_ps, rbc)
                    oh_l.append(oh)
                while done < len(extra):
                    extra[done]()
                    done += 1
                return oh_l

            # ---- pipelined schedule: attention(bk) absorbs wo(bk-1) and
            # the k projection of bk+1; wo(b3) is the only trailing phase ----
            q_sb = emit_proj_rest(0, k_done=False)
            prev_oh = None
            for bk in range(NBLK):
                extra = []
                if prev_oh is not None:
                    extra += wo_jobs(bk - 1, prev_oh, final=False)
                if bk + 1 < NBLK:
                    extra += k_mm_jobs(bk + 1)
                if bk == 0:
                    # first half of blocks 2-3's x: only 8 DMA engines get
                    # parked on long transfers, so block 1's rope swaps
                    # still find free engines
                    load_x_cols(1, 0, 8)
                prev_oh = emit_attention(bk, q_sb, extra)
                if bk + 1 < NBLK:
                    q_sb = emit_proj_rest(bk + 1, k_done=True)
                    if bk == 0:
                        # rest of pair 1, after block 1's swaps took engines;
                        # k(2) only reads c>=8 late in attention(1)
                        load_x_cols(1, 8, NC)
            for job in wo_jobs(NBLK - 1, prev_oh, final=True):
                job()

    nc.compile()
    return nc


def _prep_inputs(x, mask, freqs_cis, alibi_bias, Wq, Wk, Wv, Wo):
    """Host-side prep: partition-major packing, bf16 casts, RoPE tables,
    ALiBi bias decomposition."""
    f64 = np.float64
    idx = np.arange(HD)
    cos_full = freqs_cis[:, idx // 2]                     # [T, 128]
    sin_full = freqs_cis[:, (HD // 2) + idx // 2]         # [T, 128]
    sign = np.where(idx < HD // 2, -1.0, 1.0).astype(np.float32)
    cosT = np.ascontiguousarray(cos_full.T).astype(np.float32)          # [128, T]
    sinT = np.ascontiguousarray((sin_full * sign[None, :]).T).astype(np.float32)

    # triangle mask block: query i, key p -> 0 if i >= p else -1e9
    mtri = np.where(np.arange(P)[None, :] >= np.arange(P)[:, None],
                    0.0, -1e9).astype(np.float32)

    onesc = np.ones((P, 1), BF)
    idin = np.eye(P, dtype=np.float32)
    # rotate-half permutation as matmul lhsT: out[m] = in[(m+64)%128]
    swm = np.zeros((P, P), np.float32)
    swm[(np.arange(P) + P // 2) % P, np.arange(P)] = 1.0

    def pack(w, n):
        # [NC*P, n] -> [P, NC*n] partition-major
        return np.ascontiguousarray(
            w.reshape(-1, P, n).transpose(1, 0, 2).reshape(P, -1)).astype(BF)

    in_maps = []
    for c in range(8):
        b, gk = c // 4, c % 4
        slopes = np.array([-f64(alibi_bias[0, gk * NREP + hl, 1, 0]) for hl in range(NREP)])
        pvec = np.arange(P, dtype=f64)
        jvec = np.arange(NJ, dtype=f64)
        bkvec = np.arange(NBLK, dtype=f64)
        # cb[p, h, bk, j] = W*slope*(j*128 + p) - W*slope*(bk*512 + 511)
        cbv = (ALIBI_W * slopes[:, None, None, None]
               * (jvec[None, None, :, None] * P + pvec[None, None, None, :]
                  - (bkvec[None, :, None, None] * TB + (TB - 1))))
        cbm = np.ascontiguousarray(
            cbv.transpose(3, 0, 1, 2).reshape(P, NREP * NBLK * NJ)).astype(np.float32)
        in_maps.append({
            "xp": pack(np.ascontiguousarray(x[b].T), T),
            "wqp": pack(np.float32(SCALE) * Wq[:, gk * KVD:(gk + 1) * KVD], KVD),
            "wkp": pack(Wk[:, gk * P:(gk + 1) * P], P),
            "wvp": pack(Wv[:, gk * P:(gk + 1) * P], P),
            "wop": pack(Wo[gk * KVD:(gk + 1) * KVD, :], D),
            "cosT": cosT, "sinT": sinT,
            "cb": cbm, "mtri": mtri,
            "onesc": onesc, "idin": idin, "swm": swm,
        })
    return in_maps


def kernel(x, mask, freqs_cis, alibi_bias, Wq, Wk, Wv, Wo, _trace=False, _trace_kwargs=None):
    from concourse.bass_utils import run_bass_kernel_spmd

    if "nc" not in _cache:
        _cache["nc"] = _build()
    nc = _cache["nc"]

    in_maps = _prep_inputs(np.asarray(x, np.float32), np.asarray(mask, np.float32),
                           np.asarray(freqs_cis, np.float32), np.asarray(alibi_bias, np.float32),
                           np.asarray(Wq, np.float32), np.asarray(Wk, np.float32),
                           np.asarray(Wv, np.float32), np.asarray(Wo, np.float32))
    kw = {}
    if _trace:
        kw = dict(trace=True, **(_trace_kwargs or {}))
    res = run_bass_kernel_spmd(nc, in_maps, list(range(8)), **kw)

    full = np.zeros((B, T, D), np.float32)
    for c in range(8):
        full[c // 4] += np.asarray(res.results[c]["out"]).astype(np.float32)
    if _trace:
        _cache["last_trace"] = res
    return full
